# revision 1
# baseline (speedup 1.0000x reference)
import numpy as np
import jax
import jax.numpy as jnp

# nn_CausalLinearAttention: query (8, 512, 64, 128) f32; W* (128,128); b* (128,)
# Data-parallel over batch B=8 -> one batch element per NeuronCore (8 cores).
# Per core: chunked causal linear attention (fast_transformers style),
# feature map phi(x) = elu(x)+1, eps = 1e-6.

HEADS = 8
HEAD_DIM = 16
EPS = 1e-6
L = 512
N = 64
F = 128
C = 128          # time chunk
NC = L // C      # 4 chunks


def _per_device(xb, Wq, bq, Wk, bk, Wv, bv):
    # xb: (L, N, F) one batch element
    x = jnp.swapaxes(xb, 0, 1)                    # (N, L, F)
    q = jax.nn.elu(x @ Wq + bq) + 1.0             # (N, L, 128)
    k = jax.nn.elu(x @ Wk + bk) + 1.0
    v = x @ Wv + bv
    H, E = HEADS, HEAD_DIM
    qc = q.reshape(N, NC, C, H, E)
    kc = k.reshape(N, NC, C, H, E)
    vc = v.reshape(N, NC, C, H, E)

    # intra-chunk (diagonal blocks), causal mask incl. diagonal
    A = jnp.einsum('ncthe,ncshe->nchts', qc, kc)          # (N,NC,H,C,C)
    mask = jnp.tril(jnp.ones((C, C), dtype=x.dtype))
    Am = A * mask
    intra = jnp.einsum('nchts,ncshf->ncthf', Am, vc)      # (N,NC,C,H,E)
    den_intra = jnp.sum(Am, axis=-1)                      # (N,NC,H,C)
    den_intra = jnp.moveaxis(den_intra, 2, 3)             # (N,NC,C,H)

    # inter-chunk via exclusive cumulative KV state
    kv = jnp.einsum('ncshe,ncshf->nchef', kc, vc)         # (N,NC,H,E,E)
    S = jnp.cumsum(kv, axis=1) - kv                       # exclusive prefix
    inter = jnp.einsum('ncthe,nchef->ncthf', qc, S)       # (N,NC,C,H,E)

    ks = jnp.sum(kc, axis=2)                              # (N,NC,H,E)
    Ks = jnp.cumsum(ks, axis=1) - ks                      # exclusive prefix
    den_inter = jnp.einsum('ncthe,nche->ncth', qc, Ks)    # (N,NC,C,H)

    den = den_intra + den_inter + EPS                     # (N,NC,C,H)
    out = (intra + inter) / den[..., None]                # (N,NC,C,H,E)
    out = out.reshape(N, L, H * E)
    return jnp.swapaxes(out, 0, 1)                        # (L, N, 128)


_pmapped = None


def _get_pmapped():
    global _pmapped
    if _pmapped is None:
        _pmapped = jax.pmap(
            _per_device,
            in_axes=(0, None, None, None, None, None, None),
            devices=jax.devices()[:8],
        )
    return _pmapped


def kernel(query, Wq, bq, Wk, bk, Wv, bv):
    fn = _get_pmapped()
    out = fn(
        jnp.asarray(query, jnp.float32),
        jnp.asarray(Wq, jnp.float32), jnp.asarray(bq, jnp.float32),
        jnp.asarray(Wk, jnp.float32), jnp.asarray(bk, jnp.float32),
        jnp.asarray(Wv, jnp.float32), jnp.asarray(bv, jnp.float32),
    )
    return np.asarray(out, dtype=np.float32)
